# revision 1
# baseline (speedup 1.0000x reference)
"""CapsuleNetwork (conv->BN->relu->primary caps->squash->dynamic routing) on 8 trn2 cores.

Strategy: pure data parallel over the flattened token axis N=B*S=8192 (1024
tokens/core).  Device kernel works in "tokens-on-free" layout: every on-chip
tensor is [feature-rows (<=128 partitions), token-columns].  All contractions
(GEMMs, W_route applications, partition-group reductions and broadcasts) run
on the PE array as fp32r matmuls; the per-token bilinear products (c*p and
p*rr) run on DVE/GPSIMD; transcendentals on ACT via the single
natural_log_exp table set (rsqrt x = exp(-0.5 ln x), 1/x = exp(-ln x)).

Layouts (per 512-token tile, tokens always on the free axis):
  xT, h, praw, p:   2 chunks [128, F], rows = feature (d / oc / (i,d))
  c, exp(blog), blog, a: 4 chunks [128, F], rows = (r, i), j = 4*chunk + r
  sv, v:            4 PSUM banks [128, F], j's 32-row slot = bank j//4,
                    rows 32*(j%4)+o, o<16 real, o>=16 zero-padded
  sq/ssv/Z scales:  [32|16, F] at partition base 0

Host-side (free) prep: x is passed pre-transposed per core, BN folded into
conv1, conv k=5 center taps pre-sliced, W_route pre-packed into matmul
operand layouts (incl. zero-padding + base-partition replication), and the
final (j,o)->(o,j) output permute + junk-row drop is numpy.
"""

import sys

sys.path.insert(0, "/opt/trn_rl_repo")

import numpy as np

import concourse.bacc as bacc
import concourse.mybir as mybir
from concourse import tile
from concourse.bass_utils import run_bass_kernel_spmd

B, S, D = 4, 2048, 256
PC, PD = 32, 8
OC, OD = 16, 16
BN_EPS = 1e-5
SQ_EPS = 1e-8
NCORES = 8
NTOK = B * S
NCORE_TOK = NTOK // NCORES  # 1024

F32 = mybir.dt.float32
F32R = mybir.dt.float32r
AF = mybir.ActivationFunctionType
ALU = mybir.AluOpType

# j's whose big per-token multiplies go via ACT-evac + GPSIMD instead of DVE.
GP_JS = frozenset(j for j in range(16) if j % 4 == 3)


def r32(ap):
    return ap.bitcast(F32R)


def host_prep(conv1_w, conv1_b, bn_gamma, bn_beta, bn_mean, bn_var, pc_w, pc_b, W_route):
    """Pack all weights into the exact SBUF layouts the device kernel uses."""
    f = np.float32
    scale = (bn_gamma / np.sqrt(bn_var + BN_EPS)).astype(f)
    w1_eff = conv1_w[:, :, 2].astype(f) * scale[:, None]  # [oc, d]
    w1t = np.ascontiguousarray(w1_eff.T)  # [d, oc]
    W1T = np.concatenate([w1t[:128], w1t[128:]], axis=1)  # [128, 512] cols=(kc,oc)
    b1 = ((conv1_b - bn_mean) * scale + bn_beta).astype(f)
    B1 = np.ascontiguousarray(b1.reshape(2, 128).T)  # [128, 2]

    w2t = np.ascontiguousarray(pc_w[:, :, 2].astype(f).T)  # [oc, (i,d)]
    W2T = np.concatenate([w2t[:128], w2t[128:]], axis=1)  # [128, 512]
    B2 = np.ascontiguousarray(pc_b.astype(f).reshape(2, 128).T)  # [128, 2]

    Wr = W_route.astype(f)  # [j, i, o, d]
    tt = Wr.transpose(1, 3, 0, 2)  # [i, d, j, o]
    flat = tt.reshape(256, 16, 16)  # [(i,d), j, o]

    # WSUM [128, 2kc * 4bank * 128]: sv1 = (W/16) @ p into the 4-bank sv layout
    # bank nb rows 32r+c: j=4nb+r, value W[j,i,c,d]/16 for c<16 else 0.
    WSUM = np.zeros((128, 2 * 4 * 128), f)
    for kc in range(2):
        for nb in range(4):
            blk = np.zeros((128, 128), f)
            for r in range(4):
                blk[:, 32 * r: 32 * r + 16] = flat[kc * 128:(kc + 1) * 128, 4 * nb + r] / 16.0
            WSUM[:, (kc * 4 + nb) * 128:(kc * 4 + nb) * 128 + 128] = blk

    # WSV [128, 16j * 2m * 128]: per (j, m) an [128, 128] lhsT whose only
    # nonzero cols are 32*(j%4)+o -- the 4 j's of one bank accumulate into a
    # full M=128 matmul at dst partition 0 (col-offset dsts are illegal).
    WSV = np.zeros((128, 4096), f)
    for j in range(16):
        for m in range(2):
            base = (j * 2 + m) * 128
            WSV[:, base + 32 * (j % 4): base + 32 * (j % 4) + 16] = \
                flat[m * 128:(m + 1) * 128, j]

    # WRR [128, 16j * 2m * 128]: rows 32q+o hold W[j,i,o,d] (replicated at
    # each 32-aligned base q so lhsT base matches the sv-slot rhs base).
    wrr = tt.transpose(3, 2, 0, 1).reshape(16, 16, 256)  # [o, j, (i,d)]
    WRR = np.zeros((128, 16 * 2 * 128), f)
    for q in range(4):
        for j in range(16):
            for m in range(2):
                WRR[32 * q: 32 * q + 16, j * 256 + m * 128: j * 256 + (m + 1) * 128] = \
                    wrr[:, j, m * 128:(m + 1) * 128]

    # EAD [128, 2m * 32]: chunk m reduces d-groups into cols 16m+i_rel.
    EAD = np.zeros((128, 64), f)
    for m in range(2):
        for p in range(128):
            EAD[p, m * 32 + 16 * m + p // 8] = 1.0
    # EADX [128, (m,r) * 128]: like EAD but shifted to cols 32r+16m+i_rel so a
    # whole blog chunk (4 j's) accumulates as M=128 matmuls at dst partition 0.
    EADX = np.zeros((128, 2 * 4 * 128), f)
    for m in range(2):
        for r in range(4):
            for p in range(128):
                EADX[p, (m * 4 + r) * 128 + 32 * r + 16 * m + p // 8] = 1.0
    # ESO [128, 4nb * 16]: bank nb: rows 32r+o (o<16) -> col j_local=4nb+r.
    ESO = np.zeros((128, 64), f)
    for nb in range(4):
        for r in range(4):
            for o in range(16):
                ESO[32 * r + o, nb * 16 + 4 * nb + r] = 1.0
    # EZ [128, 32]: rows (r,i) -> col i (softmax Z: sum over the chunk's 4 j's)
    EZ = np.zeros((128, 32), f)
    for p in range(128):
        EZ[p, p % 32] = 1.0
    # ERZ [32, 128]: row i -> cols (r,i)
    ERZ = np.ascontiguousarray(EZ.T)
    # ECX [128, 2m * 128]: lhsT slice [32 @ base 32r, 128] replicated at each
    # 32-base: E[i, (i_rel,d)] = delta(i, 16m+i_rel)
    ECX = np.zeros((128, 256), f)
    for r in range(4):
        for m in range(2):
            for p in range(128):
                ECX[32 * r + (16 * m + p // 8), m * 128 + p] = 1.0
    # ESV [16, 4q * 128]: row j -> cols (r,i) of chunk q where j=4q+r
    ESV = np.zeros((16, 512), f)
    for q in range(4):
        for p in range(128):
            ESV[4 * q + p // 32, q * 128 + p] = 1.0
    # EVO [16, 4nb * 128]: row j -> bank-nb cols 32r+o (o<16), j=4nb+r
    EVO = np.zeros((16, 512), f)
    for nb in range(4):
        for r in range(4):
            for o in range(16):
                EVO[4 * nb + r, nb * 128 + 32 * r + o] = 1.0

    out = dict(W1T=W1T, B1=B1, W2T=W2T, B2=B2, WSUM=WSUM, WSV=WSV, WRR=WRR,
               EAD=EAD, EADX=EADX, ESO=ESO, EZ=EZ, ERZ=ERZ, ECX=ECX, ESV=ESV,
               EVO=EVO, EPSB=np.full((128, 1), SQ_EPS, f))
    return {k: np.ascontiguousarray(v.astype(f)) for k, v in out.items()}


WSHAPES = dict(
    W1T=[128, 512], B1=[128, 2], W2T=[128, 512], B2=[128, 2],
    WSUM=[128, 1024], WSV=[128, 4096], WRR=[128, 4096],
    EAD=[128, 64], EADX=[128, 1024], ESO=[128, 64], EZ=[128, 32], ERZ=[32, 128],
    ECX=[128, 256], ESV=[16, 512], EVO=[16, 512], EPSB=[128, 1],
)


def build_module(n_core=NCORE_TOK, F=512, reps=1, gp_js=None, stages='full'):
    """Build the per-core Bass module.  Same NEFF on all 8 cores (SPMD)."""
    NT = n_core // F
    assert NT * F == n_core
    gp_set = GP_JS if gp_js is None else frozenset(gp_js)
    nc = bacc.Bacc("TRN2", target_bir_lowering=False, debug=False, num_devices=NCORES)

    xt_d = nc.dram_tensor("xt", [256, n_core], F32R, kind="ExternalInput")
    out_d = nc.dram_tensor("out", [512, n_core], F32, kind="ExternalOutput")
    wd = {k: nc.dram_tensor(k, shp, F32R, kind="ExternalInput")
          for k, shp in WSHAPES.items()}

    with tile.TileContext(nc) as tc:
        with (
            tc.tile_pool(name="wpool", bufs=1) as wpool,
            tc.tile_pool(name="xpool", bufs=2) as xpool,
            tc.tile_pool(name="hpool", bufs=2) as hpool,
            tc.tile_pool(name="ppool", bufs=2) as ppool,
            tc.tile_pool(name="cpool", bufs=1) as cpool,
            tc.tile_pool(name="qpool", bufs=2) as qpool,
            tc.tile_pool(name="blogpool", bufs=2) as blogpool,
            tc.tile_pool(name="smpool", bufs=1) as smpool,
            tc.tile_pool(name="vpool", bufs=1) as vpool,
            tc.tile_pool(name="ps_A", bufs=(2 if F >= 512 else 4), space="PSUM") as ps_A,
        ):
            w = {}
            for k, shp in WSHAPES.items():
                w[k] = wpool.tile(shp, F32, tag=f"w_{k}", name=f"w_{k}")
                nc.sync.dma_start(r32(w[k][:]), wd[k][:])

            F2, F4 = 2 * F, 4 * F

            def mm(out_ap, lhsT_ap, rhs_ap, start=True, stop=True, tp=(0, 0)):
                nc.tensor.matmul(out_ap, r32(lhsT_ap), r32(rhs_ap), start=start,
                                 stop=stop, tile_position=tp)

            def g3(ap):
                return ap.rearrange("p (g f) -> p g f", g=4)

            def b4(ap):
                # [128, F] -> [128, 4, F] with step-0 broadcast on the group dim
                return ap.unsqueeze(1).to_broadcast((128, 4, F))

            I32 = mybir.dt.int32
            MAGIC = 0x5F3759DF

            def dve_rsqrt(y, x, sc1, sc2, P_act, Fw):
                """y = 1/sqrt(x) entirely on DVE (bit-hack seed + 2 Newton steps).
                sc1/sc2: scratch tiles.  All APs [P_act, Fw] fp32 SBUF."""
                nc.vector.tensor_scalar(sc1.bitcast(I32), x.bitcast(I32), 1, None,
                                        op0=ALU.logical_shift_right)
                nc.vector.tensor_scalar(sc2.bitcast(I32), sc1.bitcast(I32), -1, None,
                                        op0=ALU.bitwise_xor)
                nc.vector.tensor_scalar(y.bitcast(I32), sc2.bitcast(I32), MAGIC + 1,
                                        None, op0=ALU.add)
                for _ in range(2):
                    nc.vector.tensor_tensor(sc1, y, y, ALU.mult)
                    nc.vector.tensor_tensor(sc2, sc1, x, ALU.mult)
                    nc.vector.tensor_scalar(sc1, sc2, -0.5, 1.5, op0=ALU.mult,
                                            op1=ALU.add)
                    nc.vector.tensor_tensor(y, y, sc1, ALU.mult)

            def squash_scale(dst, sq_ap, P_act, Fw):
                """dst = sq/(1+sq)/sqrt(sq+eps), sq read from PSUM [P_act, Fw].
                All-DVE so the whole chain has no cross-engine hops."""
                xs = smpool.tile([128, F], F32, tag="sq_xs", name="sq_xs")
                ws = smpool.tile([128, F], F32, tag="sq_ws", name="sq_ws")
                rs = smpool.tile([128, F], F32, tag="sq_rs", name="sq_rs")
                rw = smpool.tile([128, F], F32, tag="sq_rw", name="sq_rw")
                t1 = smpool.tile([128, F], F32, tag="sq_t1", name="sq_t1")
                t2 = smpool.tile([128, F], F32, tag="sq_t2", name="sq_t2")
                a = (slice(0, P_act), slice(0, Fw))
                nc.vector.tensor_scalar(xs[a], sq_ap, SQ_EPS, None, op0=ALU.add)
                nc.vector.tensor_scalar(ws[a], sq_ap, 1.0, None, op0=ALU.add)
                dve_rsqrt(rs[a], xs[a], t1[a], t2[a], P_act, Fw)
                dve_rsqrt(rw[a], ws[a], t1[a], t2[a], P_act, Fw)
                # sq/(1+sq) = (x-eps)*rw^2 ~= x*rw^2 - eps*rw^2; use exact sq via x-eps
                nc.vector.tensor_scalar(xs[a], xs[a], -SQ_EPS, None, op0=ALU.add)
                nc.vector.tensor_tensor(t1[a], rw[a], rw[a], ALU.mult)
                nc.vector.tensor_tensor(t2[a], xs[a], t1[a], ALU.mult)
                nc.vector.tensor_tensor(dst, t2[a], rs[a], ALU.mult)

            for rep_ti in range(reps * NT):
                t_i = rep_ti % NT
                cols = slice(t_i * F, (t_i + 1) * F)
                XT2 = xpool.tile([128, F2], F32, tag="xt2", name="xt2")
                for m in range(2):
                    nc.sync.dma_start(r32(XT2[:, m * F:(m + 1) * F]),
                                      xt_d[m * 128:(m + 1) * 128, cols])

                # ---- GEMM1 + BN + relu ----
                H2 = hpool.tile([128, F2], F32, tag="h2", name="h2")
                pg = ps_A.tile([128, F4], F32, tag="A", name="pg1")
                for mc in range(2):
                    for kc in range(2):
                        mm(pg[:, mc * F:(mc + 1) * F],
                           w["W1T"][:, kc * 256 + mc * 128: kc * 256 + mc * 128 + 128],
                           XT2[:, kc * F:(kc + 1) * F], start=(kc == 0), stop=(kc == 1))
                for mc in range(2):
                    nc.scalar.activation(r32(H2[:, mc * F:(mc + 1) * F]),
                                         pg[:, mc * F:(mc + 1) * F], AF.Relu,
                                         bias=w["B1"][:, mc:mc + 1])

                # ---- GEMM2 + bias + squash(p) ----
                PRAW2 = ppool.tile([128, F2], F32, tag="praw2", name="praw2")
                pg2 = ps_A.tile([128, F4], F32, tag="A", name="pg2")
                for mc in range(2):
                    for kc in range(2):
                        mm(pg2[:, mc * F:(mc + 1) * F],
                           w["W2T"][:, kc * 256 + mc * 128: kc * 256 + mc * 128 + 128],
                           H2[:, kc * F:(kc + 1) * F], start=(kc == 0), stop=(kc == 1))
                    nc.scalar.activation(PRAW2[:, mc * F:(mc + 1) * F],
                                         pg2[:, mc * F:(mc + 1) * F], AF.Identity,
                                         bias=w["B2"][:, mc:mc + 1])
                SQT2 = qpool.tile([128, F2], F32, tag="q4a", name="sqt2")
                nc.scalar.activation(r32(SQT2[:]), PRAW2[:], AF.Square)
                pq = ps_A.tile([128, F4], F32, tag="A", name="pq")
                for m in range(2):
                    mm(pq[0:32, :F], w["EAD"][:, m * 32:(m + 1) * 32],
                       SQT2[:, m * F:(m + 1) * F], start=(m == 0), stop=(m == 1))
                SP = smpool.tile([128, F], F32, tag="s_p", name="s_p")
                squash_scale(r32(SP[:32, :F]), pq[:32, :F], 32, F)
                psx = ps_A.tile([128, F4], F32, tag="A", name="psx")
                for m in range(2):
                    mm(psx[:, m * F:(m + 1) * F], w["ECX"][:32, m * 128:(m + 1) * 128],
                       SP[:32, :F])
                P2 = ppool.tile([128, F2], F32, tag="p2", name="p2")
                nc.vector.scalar_tensor_tensor(
                    r32(P2[:]), psx[:, :F2], 1.0, PRAW2[:], op0=ALU.mult, op1=ALU.mult)

                BLOG4 = blogpool.tile([128, F4], F32, tag="blog4", name="blog4")

                for it in (1, 2, 3):
                    psv4 = ps_A.tile([128, F4], F32, tag="A", name="psv4")
                    if it == 1:
                        for g in range(4):
                            for kc in range(2):
                                mm(psv4[:, g * F:(g + 1) * F],
                                   w["WSUM"][:, (kc * 4 + g) * 128:(kc * 4 + g) * 128 + 128],
                                   P2[:, kc * F:(kc + 1) * F],
                                   start=(kc == 0), stop=(kc == 1))
                    else:
                        # softmax over j
                        EB4 = cpool.tile([128, F4], F32, tag="eb4", name="eb4")
                        nc.scalar.activation(r32(EB4[:]), BLOG4[:], AF.Exp)
                        pzx = ps_A.tile([128, F4], F32, tag="A", name="pzx")
                        for q in range(4):
                            mm(pzx[:32, :F], w["EZ"][:], EB4[:, q * F:(q + 1) * F],
                               start=(q == 0), stop=(q == 3))
                        RZ = smpool.tile([128, F], F32, tag="rz", name="rz")
                        with nc.allow_low_precision("f32r round of 1/Z"):
                            nc.vector.reciprocal(r32(RZ[:32, :F]), pzx[:32, :F])
                        mm(pzx[:, F:F2], w["ERZ"][:], RZ[:32, :F])
                        C4 = cpool.tile([128, F4], F32, tag="c4", name="c4")
                        nc.vector.scalar_tensor_tensor(
                            g3(r32(C4[:])), b4(pzx[:, F:F2]), 1.0, g3(EB4[:]),
                            op0=ALU.mult, op1=ALU.mult)
                        # q = cexp * p ; sv = WSV^T q   (4 j's per group g)
                        for g in range(4):
                            Q4 = [None, None]
                            for m in range(2):
                                cx4 = ps_A.tile([128, F4], F32, tag="A", name="cx4")
                                for r in range(4):
                                    j = 4 * g + r
                                    mm(cx4[:, r * F:(r + 1) * F],
                                       w["ECX"][r * 32:(r + 1) * 32, m * 128:(m + 1) * 128],
                                       C4[r * 32:(r + 1) * 32, g * F:(g + 1) * F],
                                       tp=(r * 32, 0))
                                Q4[m] = qpool.tile([128, F4], F32, tag=f"q4{'ab'[m]}",
                                                   name=f"q4{'ab'[m]}")
                                nc.vector.scalar_tensor_tensor(
                                    g3(r32(Q4[m][:])), g3(cx4[:]), 1.0,
                                    b4(P2[:, m * F:(m + 1) * F]),
                                    op0=ALU.mult, op1=ALU.mult)
                            for r in range(4):
                                j = 4 * g + r
                                for m in range(2):
                                    mm(psv4[:, g * F:(g + 1) * F],
                                       w["WSV"][:, (j * 2 + m) * 128:(j * 2 + m + 1) * 128],
                                       Q4[m][:, r * F:(r + 1) * F],
                                       start=(r == 0 and m == 0),
                                       stop=(r == 3 and m == 1))

                    # ---- squash(sv) ----
                    SQ24 = qpool.tile([128, F4], F32, tag="q4a", name="sq24")
                    nc.scalar.activation(r32(SQ24[:]), psv4[:], AF.Square)
                    pq2 = ps_A.tile([128, F4], F32, tag="A", name="pq2")
                    for g in range(4):
                        mm(pq2[0:16, :F], w["ESO"][:, g * 16:(g + 1) * 16],
                           SQ24[:, g * F:(g + 1) * F], start=(g == 0), stop=(g == 3))
                    SSV = smpool.tile([128, F], F32, tag="ssv", name="ssv")
                    squash_scale(r32(SSV[:16, :F]), pq2[:16, :F], 16, F)

                    if it < 3:
                        SVS4 = vpool.tile([128, F4], F32, tag="svs4", name="svs4")
                        nc.scalar.activation(r32(SVS4[:]), psv4[:], AF.Copy)
                        pa4 = ps_A.tile([128, F4], F32, tag="A", name="pa4")
                        for g in range(4):
                            for m in range(2):
                                rr4 = ps_A.tile([128, F4], F32, tag="A", name="rr4")
                                for r in range(4):
                                    j = 4 * g + r
                                    sl = 32 * r
                                    mm(rr4[:, r * F:(r + 1) * F],
                                       w["WRR"][sl:sl + 16,
                                                j * 256 + m * 128: j * 256 + (m + 1) * 128],
                                       SVS4[sl:sl + 16, g * F:(g + 1) * F], tp=(sl, 0))
                                PR4 = qpool.tile([128, F4], F32, tag=f"q4{'ab'[m]}",
                                                 name=f"pr4{'ab'[m]}")
                                nc.vector.scalar_tensor_tensor(
                                    g3(r32(PR4[:])), g3(rr4[:]), 1.0,
                                    b4(P2[:, m * F:(m + 1) * F]),
                                    op0=ALU.mult, op1=ALU.mult)
                                for r in range(4):
                                    mm(pa4[:, g * F:(g + 1) * F],
                                       w["EADX"][:, (m * 4 + r) * 128:(m * 4 + r + 1) * 128],
                                       PR4[:, r * F:(r + 1) * F],
                                       start=(m == 0 and r == 0),
                                       stop=(m == 1 and r == 3))
                        pse4 = ps_A.tile([128, F4], F32, tag="A", name="pse4")
                        for q in range(4):
                            mm(pse4[:, q * F:(q + 1) * F],
                               w["ESV"][:, q * 128:(q + 1) * 128], SSV[:16, :F])
                        SE4 = smpool.tile([128, F4], F32, tag="se4", name="se4")
                        nc.scalar.activation(SE4[:], pse4[:], AF.Copy)
                        if it == 1:
                            nc.vector.scalar_tensor_tensor(
                                BLOG4[:], pa4[:], 1.0, SE4[:],
                                op0=ALU.mult, op1=ALU.mult)
                        else:
                            TA4 = smpool.tile([128, F4], F32, tag="ta4", name="ta4")
                            nc.vector.scalar_tensor_tensor(
                                TA4[:], pa4[:], 1.0, SE4[:],
                                op0=ALU.mult, op1=ALU.mult)
                            nc.vector.tensor_tensor(BLOG4[:], BLOG4[:], TA4[:], ALU.add)
                    else:
                        pv4 = ps_A.tile([128, F4], F32, tag="A", name="pv4")
                        for g in range(4):
                            mm(pv4[:, g * F:(g + 1) * F],
                               w["EVO"][:, g * 128:(g + 1) * 128], SSV[:16, :F])
                        SEV4 = smpool.tile([128, F4], F32, tag="se4", name="sev4")
                        nc.scalar.activation(SEV4[:], pv4[:], AF.Copy)
                        V4 = vpool.tile([128, F4], F32, tag="v4", name="v4")
                        nc.vector.scalar_tensor_tensor(
                            V4[:], psv4[:], 1.0, SEV4[:], op0=ALU.mult, op1=ALU.mult)
                        for g in range(4):
                            nc.sync.dma_start(out_d[g * 128:(g + 1) * 128, cols],
                                              V4[:, g * F:(g + 1) * F])

    nc.finalize()
    return nc


_NC_CACHE = {}


def _get_nc(n_core, F, reps=1):
    key = (n_core, F, reps)
    if key not in _NC_CACHE:
        _NC_CACHE[key] = build_module(n_core, F, reps)
    return _NC_CACHE[key]


def unpack_out(out, n):
    """device out [512, n] -> v [n, j, o]"""
    v = out.reshape(4, 4, 32, n)[:, :, :16, :]  # [nb, r, o, n]
    v = v.transpose(3, 0, 1, 2).reshape(n, 16, 16)  # [n, j=4nb+r, o]
    return v


def kernel(x, conv1_w, conv1_b, bn_gamma, bn_beta, bn_mean, bn_var, pc_w, pc_b, W_route):
    x = np.asarray(x, np.float32)
    weights = host_prep(
        np.asarray(conv1_w), np.asarray(conv1_b), np.asarray(bn_gamma),
        np.asarray(bn_beta), np.asarray(bn_mean), np.asarray(bn_var),
        np.asarray(pc_w), np.asarray(pc_b), np.asarray(W_route))

    xt = np.ascontiguousarray(x.reshape(NTOK, D).T)  # [256, 8192]
    nc = _get_nc(NCORE_TOK, 512)
    in_maps = []
    for c in range(NCORES):
        m = {"xt": np.ascontiguousarray(xt[:, c * NCORE_TOK:(c + 1) * NCORE_TOK])}
        m.update(weights)
        in_maps.append(m)
    res = run_bass_kernel_spmd(nc, in_maps, list(range(NCORES)))
    v = np.concatenate(
        [unpack_out(res.results[c]["out"], NCORE_TOK) for c in range(NCORES)], axis=0)
    out = np.swapaxes(v, 1, 2).reshape(B, S, OD * OC)  # [n, o, j] flattened
    return np.ascontiguousarray(out.astype(np.float32))



# revision 3
# speedup vs baseline: 33.2250x; 33.2250x over previous
"""CapsuleNetwork (conv->BN->relu->primary caps->squash->dynamic routing) on 8 trn2 cores.

Strategy: pure data parallel over the flattened token axis N=B*S=8192 (1024
tokens/core).  Device kernel works in "tokens-on-free" layout: every on-chip
tensor is [feature-rows (<=128 partitions), token-columns].  All contractions
(GEMMs, W_route applications, partition-group reductions and broadcasts) run
on the PE array as fp32r matmuls; the per-token bilinear products (c*p and
p*rr) run on DVE/GPSIMD; transcendentals on ACT via the single
natural_log_exp table set (rsqrt x = exp(-0.5 ln x), 1/x = exp(-ln x)).

Layouts (per 512-token tile, tokens always on the free axis):
  xT, h, praw, p:   2 chunks [128, F], rows = feature (d / oc / (i,d))
  c, exp(blog), blog, a: 4 chunks [128, F], rows = (r, i), j = 4*chunk + r
  sv, v:            4 PSUM banks [128, F], j's 32-row slot = bank j//4,
                    rows 32*(j%4)+o, o<16 real, o>=16 zero-padded
  sq/ssv/Z scales:  [32|16, F] at partition base 0

Host-side (free) prep: x is passed pre-transposed per core, BN folded into
conv1, conv k=5 center taps pre-sliced, W_route pre-packed into matmul
operand layouts (incl. zero-padding + base-partition replication), and the
final (j,o)->(o,j) output permute + junk-row drop is numpy.
"""

import sys

sys.path.insert(0, "/opt/trn_rl_repo")

import numpy as np

import concourse.bacc as bacc
import concourse.mybir as mybir
from concourse import tile
from concourse.bass_utils import run_bass_kernel_spmd

B, S, D = 4, 2048, 256
PC, PD = 32, 8
OC, OD = 16, 16
BN_EPS = 1e-5
SQ_EPS = 1e-8
NCORES = 8
NTOK = B * S
NCORE_TOK = NTOK // NCORES  # 1024

F32 = mybir.dt.float32
F32R = mybir.dt.float32r
AF = mybir.ActivationFunctionType
ALU = mybir.AluOpType

# j's whose big per-token multiplies go via ACT-evac + GPSIMD instead of DVE.
GP_JS = frozenset(j for j in range(16) if j % 4 == 3)


def r32(ap):
    return ap.bitcast(F32R)


def host_prep(conv1_w, conv1_b, bn_gamma, bn_beta, bn_mean, bn_var, pc_w, pc_b, W_route):
    """Pack all weights into the exact SBUF layouts the device kernel uses."""
    f = np.float32
    scale = (bn_gamma / np.sqrt(bn_var + BN_EPS)).astype(f)
    w1_eff = conv1_w[:, :, 2].astype(f) * scale[:, None]  # [oc, d]
    w1t = np.ascontiguousarray(w1_eff.T)  # [d, oc]
    W1T = np.concatenate([w1t[:128], w1t[128:]], axis=1)  # [128, 512] cols=(kc,oc)
    b1 = ((conv1_b - bn_mean) * scale + bn_beta).astype(f)
    B1 = np.ascontiguousarray(b1.reshape(2, 128).T)  # [128, 2]

    w2t = np.ascontiguousarray(pc_w[:, :, 2].astype(f).T)  # [oc, (i,d)]
    W2T = np.concatenate([w2t[:128], w2t[128:]], axis=1)  # [128, 512]
    B2 = np.ascontiguousarray(pc_b.astype(f).reshape(2, 128).T)  # [128, 2]

    Wr = W_route.astype(f)  # [j, i, o, d]
    tt = Wr.transpose(1, 3, 0, 2)  # [i, d, j, o]
    flat = tt.reshape(256, 16, 16)  # [(i,d), j, o]

    # WSUM [128, 2kc * 4bank * 128]: sv1 = (W/16) @ p into the 4-bank sv layout
    # bank nb rows 32r+c: j=4nb+r, value W[j,i,c,d]/16 for c<16 else 0.
    WSUM = np.zeros((128, 2 * 4 * 128), f)
    for kc in range(2):
        for nb in range(4):
            blk = np.zeros((128, 128), f)
            for r in range(4):
                blk[:, 32 * r: 32 * r + 16] = flat[kc * 128:(kc + 1) * 128, 4 * nb + r] / 16.0
            WSUM[:, (kc * 4 + nb) * 128:(kc * 4 + nb) * 128 + 128] = blk

    # WSV [128, 16j * 2m * 128]: per (j, m) an [128, 128] lhsT whose only
    # nonzero cols are 32*(j%4)+o -- the 4 j's of one bank accumulate into a
    # full M=128 matmul at dst partition 0 (col-offset dsts are illegal).
    WSV = np.zeros((128, 4096), f)
    for j in range(16):
        for m in range(2):
            base = (j * 2 + m) * 128
            WSV[:, base + 32 * (j % 4): base + 32 * (j % 4) + 16] = \
                flat[m * 128:(m + 1) * 128, j]

    # WRR [128, 16j * 2m * 128]: rows 32q+o hold W[j,i,o,d] (replicated at
    # each 32-aligned base q so lhsT base matches the sv-slot rhs base).
    wrr = tt.transpose(3, 2, 0, 1).reshape(16, 16, 256)  # [o, j, (i,d)]
    WRR = np.zeros((128, 16 * 2 * 128), f)
    for q in range(4):
        for j in range(16):
            for m in range(2):
                WRR[32 * q: 32 * q + 16, j * 256 + m * 128: j * 256 + (m + 1) * 128] = \
                    wrr[:, j, m * 128:(m + 1) * 128]

    # EAD [128, 2m * 32]: chunk m reduces d-groups into cols 16m+i_rel.
    EAD = np.zeros((128, 64), f)
    for m in range(2):
        for p in range(128):
            EAD[p, m * 32 + 16 * m + p // 8] = 1.0
    # EADX [128, (m,r) * 128]: like EAD but shifted to cols 32r+16m+i_rel so a
    # whole blog chunk (4 j's) accumulates as M=128 matmuls at dst partition 0.
    EADX = np.zeros((128, 2 * 4 * 128), f)
    for m in range(2):
        for r in range(4):
            for p in range(128):
                EADX[p, (m * 4 + r) * 128 + 32 * r + 16 * m + p // 8] = 1.0
    # ESO [128, 4nb * 16]: bank nb: rows 32r+o (o<16) -> col j_local=4nb+r.
    ESO = np.zeros((128, 64), f)
    for nb in range(4):
        for r in range(4):
            for o in range(16):
                ESO[32 * r + o, nb * 16 + 4 * nb + r] = 1.0
    # EZ [128, 32]: rows (r,i) -> col i (softmax Z: sum over the chunk's 4 j's)
    EZ = np.zeros((128, 32), f)
    for p in range(128):
        EZ[p, p % 32] = 1.0
    # ERZ [32, 128]: row i -> cols (r,i)
    ERZ = np.ascontiguousarray(EZ.T)
    # ECX [128, 2m * 128]: lhsT slice [32 @ base 32r, 128] replicated at each
    # 32-base: E[i, (i_rel,d)] = delta(i, 16m+i_rel)
    ECX = np.zeros((128, 256), f)
    for r in range(4):
        for m in range(2):
            for p in range(128):
                ECX[32 * r + (16 * m + p // 8), m * 128 + p] = 1.0
    # ESV [16, 4q * 128]: row j -> cols (r,i) of chunk q where j=4q+r
    ESV = np.zeros((16, 512), f)
    for q in range(4):
        for p in range(128):
            ESV[4 * q + p // 32, q * 128 + p] = 1.0
    # EVO [16, 4nb * 128]: row j -> bank-nb cols 32r+o (o<16), j=4nb+r
    EVO = np.zeros((16, 512), f)
    for nb in range(4):
        for r in range(4):
            for o in range(16):
                EVO[4 * nb + r, nb * 128 + 32 * r + o] = 1.0

    out = dict(W1T=W1T, B1=B1, W2T=W2T, B2=B2, WSUM=WSUM, WSV=WSV, WRR=WRR,
               EAD=EAD, EADX=EADX, ESO=ESO, EZ=EZ, ERZ=ERZ, ECX=ECX, ESV=ESV,
               EVO=EVO, EPSB=np.full((128, 1), SQ_EPS, f))
    return {k: np.ascontiguousarray(v.astype(f)) for k, v in out.items()}


WSHAPES = dict(
    W1T=[128, 512], B1=[128, 2], W2T=[128, 512], B2=[128, 2],
    WSUM=[128, 1024], WSV=[128, 4096], WRR=[128, 4096],
    EAD=[128, 64], EADX=[128, 1024], ESO=[128, 64], EZ=[128, 32], ERZ=[32, 128],
    ECX=[128, 256], ESV=[16, 512], EVO=[16, 512], EPSB=[128, 1],
)


def build_module(n_core=NCORE_TOK, F=512, reps=1, gp_js=None, stages='full'):
    """Build the per-core Bass module.  Same NEFF on all 8 cores (SPMD)."""
    NT = n_core // F
    assert NT * F == n_core
    gp_set = GP_JS if gp_js is None else frozenset(gp_js)
    nc = bacc.Bacc("TRN2", target_bir_lowering=False, debug=False, num_devices=NCORES)

    xt_d = nc.dram_tensor("xt", [256, n_core], F32R, kind="ExternalInput")
    out_d = nc.dram_tensor("out", [512, n_core], F32, kind="ExternalOutput")
    wd = {k: nc.dram_tensor(k, shp, F32R, kind="ExternalInput")
          for k, shp in WSHAPES.items()}

    with tile.TileContext(nc) as tc:
        with (
            tc.tile_pool(name="wpool", bufs=1) as wpool,
            tc.tile_pool(name="xpool", bufs=2) as xpool,
            tc.tile_pool(name="hpool", bufs=2) as hpool,
            tc.tile_pool(name="ppool", bufs=2) as ppool,
            tc.tile_pool(name="cpool", bufs=1) as cpool,
            tc.tile_pool(name="qpool", bufs=2) as qpool,
            tc.tile_pool(name="blogpool", bufs=2) as blogpool,
            tc.tile_pool(name="smpool", bufs=1) as smpool,
            tc.tile_pool(name="vpool", bufs=1) as vpool,
            tc.tile_pool(name="ps_A", bufs=(2 if F >= 512 else 4), space="PSUM") as ps_A,
        ):
            w = {}
            for k, shp in WSHAPES.items():
                w[k] = wpool.tile(shp, F32, tag=f"w_{k}", name=f"w_{k}")
                nc.sync.dma_start(r32(w[k][:]), wd[k][:])

            F2, F4 = 2 * F, 4 * F

            def mm(out_ap, lhsT_ap, rhs_ap, start=True, stop=True, tp=(0, 0)):
                nc.tensor.matmul(out_ap, r32(lhsT_ap), r32(rhs_ap), start=start,
                                 stop=stop, tile_position=tp)

            def g3(ap):
                return ap.rearrange("p (g f) -> p g f", g=4)

            def b4(ap):
                # [128, F] -> [128, 4, F] with step-0 broadcast on the group dim
                return ap.unsqueeze(1).to_broadcast((128, 4, F))

            I32 = mybir.dt.int32
            MAGIC = 0x5F3759DF

            def dve_rsqrt(y, x, sc1, sc2, P_act, Fw):
                """y = 1/sqrt(x) entirely on DVE (bit-hack seed + 2 Newton steps).
                sc1/sc2: scratch tiles.  All APs [P_act, Fw] fp32 SBUF."""
                nc.vector.tensor_scalar(sc1.bitcast(I32), x.bitcast(I32), 1, None,
                                        op0=ALU.logical_shift_right)
                nc.vector.tensor_scalar(sc2.bitcast(I32), sc1.bitcast(I32), -1, None,
                                        op0=ALU.bitwise_xor)
                nc.vector.tensor_scalar(y.bitcast(I32), sc2.bitcast(I32), MAGIC + 1,
                                        None, op0=ALU.add)
                for _ in range(2):
                    nc.vector.tensor_tensor(sc1, y, y, ALU.mult)
                    nc.vector.tensor_tensor(sc2, sc1, x, ALU.mult)
                    nc.vector.tensor_scalar(sc1, sc2, -0.5, 1.5, op0=ALU.mult,
                                            op1=ALU.add)
                    nc.vector.tensor_tensor(y, y, sc1, ALU.mult)

            def squash_scale(dst, sq_ap, P_act, Fw):
                """dst = sq/(1+sq)/sqrt(sq+eps), sq read from PSUM [P_act, Fw].
                All-DVE so the whole chain has no cross-engine hops."""
                xs = smpool.tile([128, F], F32, tag="sq_xs", name="sq_xs")
                ws = smpool.tile([128, F], F32, tag="sq_ws", name="sq_ws")
                rs = smpool.tile([128, F], F32, tag="sq_rs", name="sq_rs")
                rw = smpool.tile([128, F], F32, tag="sq_rw", name="sq_rw")
                t1 = smpool.tile([128, F], F32, tag="sq_t1", name="sq_t1")
                t2 = smpool.tile([128, F], F32, tag="sq_t2", name="sq_t2")
                a = (slice(0, P_act), slice(0, Fw))
                nc.vector.tensor_scalar(xs[a], sq_ap, SQ_EPS, None, op0=ALU.add)
                nc.vector.tensor_scalar(ws[a], sq_ap, 1.0, None, op0=ALU.add)
                dve_rsqrt(rs[a], xs[a], t1[a], t2[a], P_act, Fw)
                dve_rsqrt(rw[a], ws[a], t1[a], t2[a], P_act, Fw)
                # sq/(1+sq) = (x-eps)*rw^2 ~= x*rw^2 - eps*rw^2; use exact sq via x-eps
                nc.vector.tensor_scalar(xs[a], xs[a], -SQ_EPS, None, op0=ALU.add)
                nc.vector.tensor_tensor(t1[a], rw[a], rw[a], ALU.mult)
                nc.vector.tensor_tensor(t2[a], xs[a], t1[a], ALU.mult)
                nc.vector.tensor_tensor(dst, t2[a], rs[a], ALU.mult)

            for rep_ti in range(reps * NT):
                t_i = rep_ti % NT
                cols = slice(t_i * F, (t_i + 1) * F)
                XT2 = xpool.tile([128, F2], F32, tag="xt2", name="xt2")
                for m in range(2):
                    nc.sync.dma_start(r32(XT2[:, m * F:(m + 1) * F]),
                                      xt_d[m * 128:(m + 1) * 128, cols])

                # ---- GEMM1 + BN + relu ----
                H2 = hpool.tile([128, F2], F32, tag="h2", name="h2")
                pg = ps_A.tile([128, F4], F32, tag="A", name="pg1")
                for mc in range(2):
                    for kc in range(2):
                        mm(pg[:, mc * F:(mc + 1) * F],
                           w["W1T"][:, kc * 256 + mc * 128: kc * 256 + mc * 128 + 128],
                           XT2[:, kc * F:(kc + 1) * F], start=(kc == 0), stop=(kc == 1))
                for mc in range(2):
                    nc.scalar.activation(r32(H2[:, mc * F:(mc + 1) * F]),
                                         pg[:, mc * F:(mc + 1) * F], AF.Relu,
                                         bias=w["B1"][:, mc:mc + 1])

                # ---- GEMM2 + bias + squash(p) ----
                PRAW2 = ppool.tile([128, F2], F32, tag="praw2", name="praw2")
                pg2 = ps_A.tile([128, F4], F32, tag="A", name="pg2")
                for mc in range(2):
                    for kc in range(2):
                        mm(pg2[:, mc * F:(mc + 1) * F],
                           w["W2T"][:, kc * 256 + mc * 128: kc * 256 + mc * 128 + 128],
                           H2[:, kc * F:(kc + 1) * F], start=(kc == 0), stop=(kc == 1))
                    nc.scalar.activation(PRAW2[:, mc * F:(mc + 1) * F],
                                         pg2[:, mc * F:(mc + 1) * F], AF.Identity,
                                         bias=w["B2"][:, mc:mc + 1])
                SQT2 = qpool.tile([128, F2], F32, tag="q4a", name="sqt2")
                nc.scalar.activation(r32(SQT2[:]), PRAW2[:], AF.Square)
                pq = ps_A.tile([128, F4], F32, tag="A", name="pq")
                for m in range(2):
                    mm(pq[0:32, :F], w["EAD"][:, m * 32:(m + 1) * 32],
                       SQT2[:, m * F:(m + 1) * F], start=(m == 0), stop=(m == 1))
                SP = smpool.tile([128, F], F32, tag="s_p", name="s_p")
                squash_scale(r32(SP[:32, :F]), pq[:32, :F], 32, F)
                psx = ps_A.tile([128, F4], F32, tag="A", name="psx")
                for m in range(2):
                    mm(psx[:, m * F:(m + 1) * F], w["ECX"][:32, m * 128:(m + 1) * 128],
                       SP[:32, :F])
                P2 = ppool.tile([128, F2], F32, tag="p2", name="p2")
                nc.vector.scalar_tensor_tensor(
                    r32(P2[:]), psx[:, :F2], 1.0, PRAW2[:], op0=ALU.mult, op1=ALU.mult)

                BLOG4 = blogpool.tile([128, F4], F32, tag="blog4", name="blog4")

                for it in (1, 2, 3):
                    psv4 = ps_A.tile([128, F4], F32, tag="A", name="psv4")
                    if it == 1:
                        for g in range(4):
                            for kc in range(2):
                                mm(psv4[:, g * F:(g + 1) * F],
                                   w["WSUM"][:, (kc * 4 + g) * 128:(kc * 4 + g) * 128 + 128],
                                   P2[:, kc * F:(kc + 1) * F],
                                   start=(kc == 0), stop=(kc == 1))
                    else:
                        # softmax over j
                        EB4 = cpool.tile([128, F4], F32, tag="eb4", name="eb4")
                        nc.scalar.activation(r32(EB4[:]), BLOG4[:], AF.Exp)
                        pzx = ps_A.tile([128, F4], F32, tag="A", name="pzx")
                        for q in range(4):
                            mm(pzx[:32, :F], w["EZ"][:], EB4[:, q * F:(q + 1) * F],
                               start=(q == 0), stop=(q == 3))
                        RZ = smpool.tile([128, F], F32, tag="rz", name="rz")
                        with nc.allow_low_precision("f32r round of 1/Z"):
                            nc.vector.reciprocal(r32(RZ[:32, :F]), pzx[:32, :F])
                        mm(pzx[:, F:F2], w["ERZ"][:], RZ[:32, :F])
                        C4 = cpool.tile([128, F4], F32, tag="c4", name="c4")
                        nc.vector.scalar_tensor_tensor(
                            g3(r32(C4[:])), b4(pzx[:, F:F2]), 1.0, g3(EB4[:]),
                            op0=ALU.mult, op1=ALU.mult)
                        # q = cexp * p ; sv = WSV^T q   (4 j's per group g)
                        for g in range(4):
                            Q4 = [None, None]
                            for m in range(2):
                                cx4 = ps_A.tile([128, F4], F32, tag="A", name="cx4")
                                for r in range(4):
                                    j = 4 * g + r
                                    mm(cx4[:, r * F:(r + 1) * F],
                                       w["ECX"][r * 32:(r + 1) * 32, m * 128:(m + 1) * 128],
                                       C4[r * 32:(r + 1) * 32, g * F:(g + 1) * F],
                                       tp=(r * 32, 0))
                                Q4[m] = qpool.tile([128, F4], F32, tag=f"q4{'ab'[m]}",
                                                   name=f"q4{'ab'[m]}")
                                nc.vector.scalar_tensor_tensor(
                                    g3(r32(Q4[m][:])), g3(cx4[:]), 1.0,
                                    b4(P2[:, m * F:(m + 1) * F]),
                                    op0=ALU.mult, op1=ALU.mult)
                            for r in range(4):
                                j = 4 * g + r
                                for m in range(2):
                                    mm(psv4[:, g * F:(g + 1) * F],
                                       w["WSV"][:, (j * 2 + m) * 128:(j * 2 + m + 1) * 128],
                                       Q4[m][:, r * F:(r + 1) * F],
                                       start=(r == 0 and m == 0),
                                       stop=(r == 3 and m == 1))

                    # ---- squash(sv) ----
                    SQ24 = qpool.tile([128, F4], F32, tag="q4a", name="sq24")
                    nc.scalar.activation(r32(SQ24[:]), psv4[:], AF.Square)
                    pq2 = ps_A.tile([128, F4], F32, tag="A", name="pq2")
                    for g in range(4):
                        mm(pq2[0:16, :F], w["ESO"][:, g * 16:(g + 1) * 16],
                           SQ24[:, g * F:(g + 1) * F], start=(g == 0), stop=(g == 3))
                    SSV = smpool.tile([128, F], F32, tag="ssv", name="ssv")
                    squash_scale(r32(SSV[:16, :F]), pq2[:16, :F], 16, F)

                    if it < 3:
                        SVS4 = vpool.tile([128, F4], F32, tag="svs4", name="svs4")
                        nc.scalar.activation(r32(SVS4[:]), psv4[:], AF.Copy)
                        pa4 = ps_A.tile([128, F4], F32, tag="A", name="pa4")
                        for g in range(4):
                            for m in range(2):
                                rr4 = ps_A.tile([128, F4], F32, tag="A", name="rr4")
                                for r in range(4):
                                    j = 4 * g + r
                                    sl = 32 * r
                                    mm(rr4[:, r * F:(r + 1) * F],
                                       w["WRR"][sl:sl + 16,
                                                j * 256 + m * 128: j * 256 + (m + 1) * 128],
                                       SVS4[sl:sl + 16, g * F:(g + 1) * F], tp=(sl, 0))
                                PR4 = qpool.tile([128, F4], F32, tag=f"q4{'ab'[m]}",
                                                 name=f"pr4{'ab'[m]}")
                                nc.vector.scalar_tensor_tensor(
                                    g3(r32(PR4[:])), g3(rr4[:]), 1.0,
                                    b4(P2[:, m * F:(m + 1) * F]),
                                    op0=ALU.mult, op1=ALU.mult)
                                for r in range(4):
                                    mm(pa4[:, g * F:(g + 1) * F],
                                       w["EADX"][:, (m * 4 + r) * 128:(m * 4 + r + 1) * 128],
                                       PR4[:, r * F:(r + 1) * F],
                                       start=(m == 0 and r == 0),
                                       stop=(m == 1 and r == 3))
                        pse4 = ps_A.tile([128, F4], F32, tag="A", name="pse4")
                        for q in range(4):
                            mm(pse4[:, q * F:(q + 1) * F],
                               w["ESV"][:, q * 128:(q + 1) * 128], SSV[:16, :F])
                        SE4 = smpool.tile([128, F4], F32, tag="se4", name="se4")
                        nc.scalar.activation(SE4[:], pse4[:], AF.Copy)
                        if it == 1:
                            nc.vector.scalar_tensor_tensor(
                                BLOG4[:], pa4[:], 1.0, SE4[:],
                                op0=ALU.mult, op1=ALU.mult)
                        else:
                            TA4 = smpool.tile([128, F4], F32, tag="ta4", name="ta4")
                            nc.vector.scalar_tensor_tensor(
                                TA4[:], pa4[:], 1.0, SE4[:],
                                op0=ALU.mult, op1=ALU.mult)
                            nc.vector.tensor_tensor(BLOG4[:], BLOG4[:], TA4[:], ALU.add)
                    else:
                        pv4 = ps_A.tile([128, F4], F32, tag="A", name="pv4")
                        for g in range(4):
                            mm(pv4[:, g * F:(g + 1) * F],
                               w["EVO"][:, g * 128:(g + 1) * 128], SSV[:16, :F])
                        SEV4 = smpool.tile([128, F4], F32, tag="se4", name="sev4")
                        nc.scalar.activation(SEV4[:], pv4[:], AF.Copy)
                        V4 = vpool.tile([128, F4], F32, tag="v4", name="v4")
                        nc.vector.scalar_tensor_tensor(
                            V4[:], psv4[:], 1.0, SEV4[:], op0=ALU.mult, op1=ALU.mult)
                        for g in range(4):
                            nc.sync.dma_start(out_d[g * 128:(g + 1) * 128, cols],
                                              V4[:, g * F:(g + 1) * F])

    nc.finalize()
    return nc


_NC_CACHE = {}


def _get_nc(n_core, F, reps=1):
    key = (n_core, F, reps)
    if key not in _NC_CACHE:
        _NC_CACHE[key] = build_module(n_core, F, reps)
    return _NC_CACHE[key]


_JIT_CACHE = {}


def get_jitted(nc):
    """Compile nc ONCE into a reusable 8-core jitted executable.

    run_bass_kernel_spmd re-creates (and re-compiles) a fresh jax.jit closure
    on every call, so each invocation pays XLA compile + NEFF reload on top of
    the actual execution.  Building the jitted callable once and reusing it
    makes repeat kernel() calls transfer+execute only.
    """
    if id(nc) in _JIT_CACHE:
        return _JIT_CACHE[id(nc)]
    import jax
    from jax.sharding import Mesh, PartitionSpec
    from jax.experimental.shard_map import shard_map
    from concourse import bass2jax
    from concourse.bass2jax import _bass_exec_p, install_neuronx_cc_hook

    install_neuronx_cc_hook()
    partition_name = nc.partition_id_tensor.name if nc.partition_id_tensor else None
    in_names, out_names, out_avals, out_shapes = [], [], [], []
    for alloc in nc.m.functions[0].allocations:
        if not isinstance(alloc, mybir.MemoryLocationSet):
            continue
        name = alloc.memorylocations[0].name
        if alloc.kind == "ExternalInput":
            if name != partition_name:
                in_names.append(name)
        elif alloc.kind == "ExternalOutput":
            shape = tuple(alloc.tensor_shape)
            dtype = mybir.dt.np(alloc.dtype)
            out_names.append(name)
            out_shapes.append((shape, dtype))
            out_avals.append(jax.core.ShapedArray(shape, dtype))
    n_params = len(in_names)
    all_in = list(in_names) + list(out_names)
    if partition_name is not None:
        all_in.append(partition_name)

    def _body(*args):
        operands = list(args)
        if partition_name is not None:
            operands.append(bass2jax.partition_id_tensor())
        outs = _bass_exec_p.bind(
            *operands, out_avals=tuple(out_avals), in_names=tuple(all_in),
            out_names=tuple(out_names), lowering_input_output_aliases=(),
            sim_require_finite=True, sim_require_nnan=True, nc=nc)
        return tuple(outs)

    donate = tuple(range(n_params, n_params + len(out_names)))
    devices = jax.devices()[:NCORES]
    mesh = Mesh(np.asarray(devices), ("core",))
    specs = (PartitionSpec("core"),) * (n_params + len(out_names))
    out_specs = (PartitionSpec("core"),) * len(out_names)
    jf = jax.jit(
        shard_map(_body, mesh=mesh, in_specs=specs, out_specs=out_specs,
                  check_rep=False),
        donate_argnums=donate, keep_unused=True)
    entry = dict(jf=jf, in_names=in_names, out_names=out_names,
                 out_shapes=out_shapes)
    _JIT_CACHE[id(nc)] = entry
    return entry


def run_jitted(nc, in_maps):
    """Execute via the cached jitted callable; returns {name: [per-core]}."""
    ent = get_jitted(nc)
    n = NCORES
    concat_in = [np.concatenate([np.asarray(in_maps[c][k]) for c in range(n)], axis=0)
                 for k in ent["in_names"]]
    zeros = [np.zeros((n * s[0], *s[1:]), d) for (s, d) in ent["out_shapes"]]
    out_arrs = ent["jf"](*concat_in, *zeros)
    res = {}
    for i, name in enumerate(ent["out_names"]):
        shape, _ = ent["out_shapes"][i]
        res[name] = np.asarray(out_arrs[i]).reshape(n, *shape)
    return res


def unpack_out(out, n):
    """device out [512, n] -> v [n, j, o]"""
    v = out.reshape(4, 4, 32, n)[:, :, :16, :]  # [nb, r, o, n]
    v = v.transpose(3, 0, 1, 2).reshape(n, 16, 16)  # [n, j=4nb+r, o]
    return v


def kernel(x, conv1_w, conv1_b, bn_gamma, bn_beta, bn_mean, bn_var, pc_w, pc_b, W_route):
    x = np.asarray(x, np.float32)
    weights = host_prep(
        np.asarray(conv1_w), np.asarray(conv1_b), np.asarray(bn_gamma),
        np.asarray(bn_beta), np.asarray(bn_mean), np.asarray(bn_var),
        np.asarray(pc_w), np.asarray(pc_b), np.asarray(W_route))

    xt = np.ascontiguousarray(x.reshape(NTOK, D).T)  # [256, 8192]
    nc = _get_nc(NCORE_TOK, 512)
    in_maps = []
    for c in range(NCORES):
        m = {"xt": np.ascontiguousarray(xt[:, c * NCORE_TOK:(c + 1) * NCORE_TOK])}
        m.update(weights)
        in_maps.append(m)
    res = run_jitted(nc, in_maps)
    v = np.concatenate(
        [unpack_out(res["out"][c], NCORE_TOK) for c in range(NCORES)], axis=0)
    out = np.swapaxes(v, 1, 2).reshape(B, S, OD * OC)  # [n, o, j] flattened
    return np.ascontiguousarray(out.astype(np.float32))



# revision 4
# speedup vs baseline: 7297.6367x; 219.6432x over previous
"""CapsuleNetwork (conv->BN->relu->primary caps->squash->dynamic routing) on 8 trn2 cores.

Strategy: pure data parallel over the flattened token axis N=B*S=8192 (1024
tokens/core).  Device kernel works in "tokens-on-free" layout: every on-chip
tensor is [feature-rows (<=128 partitions), token-columns].  All contractions
(GEMMs, W_route applications, partition-group reductions and broadcasts) run
on the PE array as fp32r matmuls; the per-token bilinear products (c*p and
p*rr) run on DVE/GPSIMD; transcendentals on ACT via the single
natural_log_exp table set (rsqrt x = exp(-0.5 ln x), 1/x = exp(-ln x)).

Layouts (per 512-token tile, tokens always on the free axis):
  xT, h, praw, p:   2 chunks [128, F], rows = feature (d / oc / (i,d))
  c, exp(blog), blog, a: 4 chunks [128, F], rows = (r, i), j = 4*chunk + r
  sv, v:            4 PSUM banks [128, F], j's 32-row slot = bank j//4,
                    rows 32*(j%4)+o, o<16 real, o>=16 zero-padded
  sq/ssv/Z scales:  [32|16, F] at partition base 0

Host-side (free) prep: x is passed pre-transposed per core, BN folded into
conv1, conv k=5 center taps pre-sliced, W_route pre-packed into matmul
operand layouts (incl. zero-padding + base-partition replication), and the
final (j,o)->(o,j) output permute + junk-row drop is numpy.
"""

import sys

sys.path.insert(0, "/opt/trn_rl_repo")

import numpy as np

import concourse.bacc as bacc
import concourse.mybir as mybir
from concourse import tile
from concourse.bass_utils import run_bass_kernel_spmd

B, S, D = 4, 2048, 256
PC, PD = 32, 8
OC, OD = 16, 16
BN_EPS = 1e-5
SQ_EPS = 1e-8
NCORES = 8
NTOK = B * S
NCORE_TOK = NTOK // NCORES  # 1024

F32 = mybir.dt.float32
F32R = mybir.dt.float32r
AF = mybir.ActivationFunctionType
ALU = mybir.AluOpType

# j's whose big per-token multiplies go via ACT-evac + GPSIMD instead of DVE.
GP_JS = frozenset(j for j in range(16) if j % 4 == 3)


def r32(ap):
    return ap.bitcast(F32R)


def host_prep(conv1_w, conv1_b, bn_gamma, bn_beta, bn_mean, bn_var, pc_w, pc_b, W_route):
    """Pack all weights into the exact SBUF layouts the device kernel uses."""
    f = np.float32
    scale = (bn_gamma / np.sqrt(bn_var + BN_EPS)).astype(f)
    w1_eff = conv1_w[:, :, 2].astype(f) * scale[:, None]  # [oc, d]
    w1t = np.ascontiguousarray(w1_eff.T)  # [d, oc]
    W1T = np.concatenate([w1t[:128], w1t[128:]], axis=1)  # [128, 512] cols=(kc,oc)
    b1 = ((conv1_b - bn_mean) * scale + bn_beta).astype(f)
    B1 = np.ascontiguousarray(b1.reshape(2, 128).T)  # [128, 2]

    w2t = np.ascontiguousarray(pc_w[:, :, 2].astype(f).T)  # [oc, (i,d)]
    W2T = np.concatenate([w2t[:128], w2t[128:]], axis=1)  # [128, 512]
    B2 = np.ascontiguousarray(pc_b.astype(f).reshape(2, 128).T)  # [128, 2]

    Wr = W_route.astype(f)  # [j, i, o, d]
    tt = Wr.transpose(1, 3, 0, 2)  # [i, d, j, o]
    flat = tt.reshape(256, 16, 16)  # [(i,d), j, o]

    # WSUM [128, 2kc * 4bank * 128]: sv1 = (W/16) @ p into the 4-bank sv layout
    # bank nb rows 32r+c: j=4nb+r, value W[j,i,c,d]/16 for c<16 else 0.
    WSUM = np.zeros((128, 2 * 4 * 128), f)
    for kc in range(2):
        for nb in range(4):
            blk = np.zeros((128, 128), f)
            for r in range(4):
                blk[:, 32 * r: 32 * r + 16] = flat[kc * 128:(kc + 1) * 128, 4 * nb + r] / 16.0
            WSUM[:, (kc * 4 + nb) * 128:(kc * 4 + nb) * 128 + 128] = blk

    # WSV [128, 16j * 2m * 128]: per (j, m) an [128, 128] lhsT whose only
    # nonzero cols are 32*(j%4)+o -- the 4 j's of one bank accumulate into a
    # full M=128 matmul at dst partition 0 (col-offset dsts are illegal).
    WSV = np.zeros((128, 4096), f)
    for j in range(16):
        for m in range(2):
            base = (j * 2 + m) * 128
            WSV[:, base + 32 * (j % 4): base + 32 * (j % 4) + 16] = \
                flat[m * 128:(m + 1) * 128, j]

    # WRR [128, 16j * 2m * 128]: rows 32q+o hold W[j,i,o,d] (replicated at
    # each 32-aligned base q so lhsT base matches the sv-slot rhs base).
    wrr = tt.transpose(3, 2, 0, 1).reshape(16, 16, 256)  # [o, j, (i,d)]
    WRR = np.zeros((128, 16 * 2 * 128), f)
    for q in range(4):
        for j in range(16):
            for m in range(2):
                WRR[32 * q: 32 * q + 16, j * 256 + m * 128: j * 256 + (m + 1) * 128] = \
                    wrr[:, j, m * 128:(m + 1) * 128]

    # EAD [128, 2m * 32]: chunk m reduces d-groups into cols 16m+i_rel.
    EAD = np.zeros((128, 64), f)
    for m in range(2):
        for p in range(128):
            EAD[p, m * 32 + 16 * m + p // 8] = 1.0
    # EADX [128, (m,r) * 128]: like EAD but shifted to cols 32r+16m+i_rel so a
    # whole blog chunk (4 j's) accumulates as M=128 matmuls at dst partition 0.
    EADX = np.zeros((128, 2 * 4 * 128), f)
    for m in range(2):
        for r in range(4):
            for p in range(128):
                EADX[p, (m * 4 + r) * 128 + 32 * r + 16 * m + p // 8] = 1.0
    # ESO [128, 4nb * 16]: bank nb: rows 32r+o (o<16) -> col j_local=4nb+r.
    ESO = np.zeros((128, 64), f)
    for nb in range(4):
        for r in range(4):
            for o in range(16):
                ESO[32 * r + o, nb * 16 + 4 * nb + r] = 1.0
    # EZ [128, 32]: rows (r,i) -> col i (softmax Z: sum over the chunk's 4 j's)
    EZ = np.zeros((128, 32), f)
    for p in range(128):
        EZ[p, p % 32] = 1.0
    # ERZ [32, 128]: row i -> cols (r,i)
    ERZ = np.ascontiguousarray(EZ.T)
    # ECX [128, 2m * 128]: lhsT slice [32 @ base 32r, 128] replicated at each
    # 32-base: E[i, (i_rel,d)] = delta(i, 16m+i_rel)
    ECX = np.zeros((128, 256), f)
    for r in range(4):
        for m in range(2):
            for p in range(128):
                ECX[32 * r + (16 * m + p // 8), m * 128 + p] = 1.0
    # ESV [16, 4q * 128]: row j -> cols (r,i) of chunk q where j=4q+r
    ESV = np.zeros((16, 512), f)
    for q in range(4):
        for p in range(128):
            ESV[4 * q + p // 32, q * 128 + p] = 1.0
    # EVO [16, 4nb * 128]: row j -> bank-nb cols 32r+o (o<16), j=4nb+r
    EVO = np.zeros((16, 512), f)
    for nb in range(4):
        for r in range(4):
            for o in range(16):
                EVO[4 * nb + r, nb * 128 + 32 * r + o] = 1.0

    out = dict(W1T=W1T, B1=B1, W2T=W2T, B2=B2, WSUM=WSUM, WSV=WSV, WRR=WRR,
               EAD=EAD, EADX=EADX, ESO=ESO, EZ=EZ, ERZ=ERZ, ECX=ECX, ESV=ESV,
               EVO=EVO, EPSB=np.full((128, 1), SQ_EPS, f))
    return {k: np.ascontiguousarray(v.astype(f)) for k, v in out.items()}


WSHAPES = dict(
    W1T=[128, 512], B1=[128, 2], W2T=[128, 512], B2=[128, 2],
    WSUM=[128, 1024], WSV=[128, 4096], WRR=[128, 4096],
    EAD=[128, 64], EADX=[128, 1024], ESO=[128, 64], EZ=[128, 32], ERZ=[32, 128],
    ECX=[128, 256], ESV=[16, 512], EVO=[16, 512], EPSB=[128, 1],
)


def build_module(n_core=NCORE_TOK, F=512, reps=1, gp_js=None, stages='full'):
    """Build the per-core Bass module.  Same NEFF on all 8 cores (SPMD)."""
    NT = n_core // F
    assert NT * F == n_core
    gp_set = GP_JS if gp_js is None else frozenset(gp_js)
    nc = bacc.Bacc("TRN2", target_bir_lowering=False, debug=False, num_devices=NCORES)

    xt_d = nc.dram_tensor("xt", [256, n_core], F32R, kind="ExternalInput")
    out_d = nc.dram_tensor("out", [512, n_core], F32, kind="ExternalOutput")
    wd = {k: nc.dram_tensor(k, shp, F32R, kind="ExternalInput")
          for k, shp in WSHAPES.items()}

    with tile.TileContext(nc) as tc:
        with (
            tc.tile_pool(name="wpool", bufs=1) as wpool,
            tc.tile_pool(name="xpool", bufs=2) as xpool,
            tc.tile_pool(name="hpool", bufs=2) as hpool,
            tc.tile_pool(name="ppool", bufs=2) as ppool,
            tc.tile_pool(name="cpool", bufs=1) as cpool,
            tc.tile_pool(name="qpool", bufs=2) as qpool,
            tc.tile_pool(name="blogpool", bufs=2) as blogpool,
            tc.tile_pool(name="smpool", bufs=1) as smpool,
            tc.tile_pool(name="vpool", bufs=1) as vpool,
            tc.tile_pool(name="ps_A", bufs=(2 if F >= 512 else 4), space="PSUM") as ps_A,
        ):
            w = {}
            for k, shp in WSHAPES.items():
                w[k] = wpool.tile(shp, F32, tag=f"w_{k}", name=f"w_{k}")
                nc.sync.dma_start(r32(w[k][:]), wd[k][:])

            F2, F4 = 2 * F, 4 * F

            def mm(out_ap, lhsT_ap, rhs_ap, start=True, stop=True, tp=(0, 0)):
                nc.tensor.matmul(out_ap, r32(lhsT_ap), r32(rhs_ap), start=start,
                                 stop=stop, tile_position=tp)

            def g3(ap):
                return ap.rearrange("p (g f) -> p g f", g=4)

            def b4(ap):
                # [128, F] -> [128, 4, F] with step-0 broadcast on the group dim
                return ap.unsqueeze(1).to_broadcast((128, 4, F))

            I32 = mybir.dt.int32
            MAGIC = 0x5F3759DF

            def dve_rsqrt(y, x, sc1, sc2, P_act, Fw, newton=1):
                """y = 1/sqrt(x) entirely on DVE (bit-hack seed + Newton steps).
                sc1/sc2: scratch tiles.  All APs [P_act, Fw] fp32 SBUF."""
                nc.vector.tensor_scalar(sc1.bitcast(I32), x.bitcast(I32), 1, None,
                                        op0=ALU.logical_shift_right)
                nc.vector.tensor_scalar(sc2.bitcast(I32), sc1.bitcast(I32), -1, None,
                                        op0=ALU.bitwise_xor)
                nc.vector.tensor_scalar(y.bitcast(I32), sc2.bitcast(I32), MAGIC + 1,
                                        None, op0=ALU.add)
                for _ in range(newton):
                    nc.vector.tensor_tensor(sc1, y, y, ALU.mult)
                    nc.vector.tensor_tensor(sc2, sc1, x, ALU.mult)
                    nc.vector.tensor_scalar(sc1, sc2, -0.5, 1.5, op0=ALU.mult,
                                            op1=ALU.add)
                    nc.vector.tensor_tensor(y, y, sc1, ALU.mult)

            def squash_scale(dst, sq_ap, P_act, Fw):
                """dst = sq/(1+sq)/sqrt(sq+eps), sq read from PSUM [P_act, Fw].

                sq/(1+sq) = 1 - 1/(1+sq) via the native DVE reciprocal, and
                rsqrt(sq+eps) via bit-hack seed + 2 Newton steps: 16 DVE ops
                (the old dual-dve_rsqrt version was 28)."""
                xs = smpool.tile([128, F], F32, tag="sq_xs", name="sq_xs")
                ws = smpool.tile([128, F], F32, tag="sq_ws", name="sq_ws")
                rs = smpool.tile([128, F], F32, tag="sq_rs", name="sq_rs")
                rw = smpool.tile([128, F], F32, tag="sq_rw", name="sq_rw")
                t1 = smpool.tile([128, F], F32, tag="sq_t1", name="sq_t1")
                t2 = smpool.tile([128, F], F32, tag="sq_t2", name="sq_t2")
                a = (slice(0, P_act), slice(0, Fw))
                nc.vector.tensor_scalar(xs[a], sq_ap, SQ_EPS, None, op0=ALU.add)
                nc.vector.tensor_scalar(ws[a], sq_ap, 1.0, None, op0=ALU.add)
                dve_rsqrt(rs[a], xs[a], t1[a], t2[a], P_act, Fw, newton=2)
                with nc.allow_low_precision("squash 1/(1+sq)"):
                    nc.vector.reciprocal(rw[a], ws[a])
                # t1 = 1 - rw = sq/(1+sq)
                nc.vector.tensor_scalar(t1[a], rw[a], -1.0, 1.0, op0=ALU.mult,
                                        op1=ALU.add)
                nc.vector.tensor_tensor(dst, t1[a], rs[a], ALU.mult)

            for rep_ti in range(reps * NT):
                t_i = rep_ti % NT
                cols = slice(t_i * F, (t_i + 1) * F)
                XT2 = xpool.tile([128, F2], F32, tag="xt2", name="xt2")
                for m in range(2):
                    nc.sync.dma_start(r32(XT2[:, m * F:(m + 1) * F]),
                                      xt_d[m * 128:(m + 1) * 128, cols])

                # ---- GEMM1 + BN + relu ----
                H2 = hpool.tile([128, F2], F32, tag="h2", name="h2")
                pg = ps_A.tile([128, F4], F32, tag="A", name="pg1")
                for mc in range(2):
                    for kc in range(2):
                        mm(pg[:, mc * F:(mc + 1) * F],
                           w["W1T"][:, kc * 256 + mc * 128: kc * 256 + mc * 128 + 128],
                           XT2[:, kc * F:(kc + 1) * F], start=(kc == 0), stop=(kc == 1))
                for mc in range(2):
                    nc.scalar.activation(r32(H2[:, mc * F:(mc + 1) * F]),
                                         pg[:, mc * F:(mc + 1) * F], AF.Relu,
                                         bias=w["B1"][:, mc:mc + 1])

                # ---- GEMM2 + bias + squash(p) ----
                PRAW2 = ppool.tile([128, F2], F32, tag="praw2", name="praw2")
                pg2 = ps_A.tile([128, F4], F32, tag="A", name="pg2")
                for mc in range(2):
                    for kc in range(2):
                        mm(pg2[:, mc * F:(mc + 1) * F],
                           w["W2T"][:, kc * 256 + mc * 128: kc * 256 + mc * 128 + 128],
                           H2[:, kc * F:(kc + 1) * F], start=(kc == 0), stop=(kc == 1))
                    nc.scalar.activation(PRAW2[:, mc * F:(mc + 1) * F],
                                         pg2[:, mc * F:(mc + 1) * F], AF.Identity,
                                         bias=w["B2"][:, mc:mc + 1])
                SQT2 = qpool.tile([128, F2], F32, tag="q4a", name="sqt2")
                nc.scalar.activation(r32(SQT2[:]), PRAW2[:], AF.Square)
                pq = ps_A.tile([128, F4], F32, tag="A", name="pq")
                for m in range(2):
                    mm(pq[0:32, :F], w["EAD"][:, m * 32:(m + 1) * 32],
                       SQT2[:, m * F:(m + 1) * F], start=(m == 0), stop=(m == 1))
                SP = smpool.tile([128, F], F32, tag="s_p", name="s_p")
                squash_scale(r32(SP[:32, :F]), pq[:32, :F], 32, F)
                psx = ps_A.tile([128, F4], F32, tag="A", name="psx")
                for m in range(2):
                    mm(psx[:, m * F:(m + 1) * F], w["ECX"][:32, m * 128:(m + 1) * 128],
                       SP[:32, :F])
                P2 = ppool.tile([128, F2], F32, tag="p2", name="p2")
                nc.vector.scalar_tensor_tensor(
                    r32(P2[:]), psx[:, :F2], 1.0, PRAW2[:], op0=ALU.mult, op1=ALU.mult)

                BLOG4 = blogpool.tile([128, F4], F32, tag="blog4", name="blog4")

                for it in (1, 2, 3):
                    psv4 = ps_A.tile([128, F4], F32, tag="A", name="psv4")
                    if it == 1:
                        for g in range(4):
                            for kc in range(2):
                                mm(psv4[:, g * F:(g + 1) * F],
                                   w["WSUM"][:, (kc * 4 + g) * 128:(kc * 4 + g) * 128 + 128],
                                   P2[:, kc * F:(kc + 1) * F],
                                   start=(kc == 0), stop=(kc == 1))
                    else:
                        # softmax over j
                        EB4 = cpool.tile([128, F4], F32, tag="eb4", name="eb4")
                        nc.scalar.activation(r32(EB4[:]), BLOG4[:], AF.Exp)
                        pzx = ps_A.tile([128, F4], F32, tag="A", name="pzx")
                        for q in range(4):
                            mm(pzx[:32, :F], w["EZ"][:], EB4[:, q * F:(q + 1) * F],
                               start=(q == 0), stop=(q == 3))
                        RZ = smpool.tile([128, F], F32, tag="rz", name="rz")
                        with nc.allow_low_precision("f32r round of 1/Z"):
                            nc.vector.reciprocal(r32(RZ[:32, :F]), pzx[:32, :F])
                        mm(pzx[:, F:F2], w["ERZ"][:], RZ[:32, :F])
                        C4 = cpool.tile([128, F4], F32, tag="c4", name="c4")
                        nc.vector.scalar_tensor_tensor(
                            g3(r32(C4[:])), b4(pzx[:, F:F2]), 1.0, g3(EB4[:]),
                            op0=ALU.mult, op1=ALU.mult)
                        # q = cexp * p ; sv = WSV^T q   (4 j's per group g)
                        for g in range(4):
                            Q4 = [None, None]
                            for m in range(2):
                                cx4 = ps_A.tile([128, F4], F32, tag="A", name="cx4")
                                for r in range(4):
                                    j = 4 * g + r
                                    mm(cx4[:, r * F:(r + 1) * F],
                                       w["ECX"][r * 32:(r + 1) * 32, m * 128:(m + 1) * 128],
                                       C4[r * 32:(r + 1) * 32, g * F:(g + 1) * F],
                                       tp=(r * 32, 0))
                                Q4[m] = qpool.tile([128, F4], F32, tag=f"q4{'ab'[m]}",
                                                   name=f"q4{'ab'[m]}")
                                nc.vector.scalar_tensor_tensor(
                                    g3(r32(Q4[m][:])), g3(cx4[:]), 1.0,
                                    b4(P2[:, m * F:(m + 1) * F]),
                                    op0=ALU.mult, op1=ALU.mult)
                            for r in range(4):
                                j = 4 * g + r
                                for m in range(2):
                                    mm(psv4[:, g * F:(g + 1) * F],
                                       w["WSV"][:, (j * 2 + m) * 128:(j * 2 + m + 1) * 128],
                                       Q4[m][:, r * F:(r + 1) * F],
                                       start=(r == 0 and m == 0),
                                       stop=(r == 3 and m == 1))

                    # ---- squash(sv) ----
                    SQ24 = qpool.tile([128, F4], F32, tag="q4a", name="sq24")
                    nc.scalar.activation(r32(SQ24[:]), psv4[:], AF.Square)
                    pq2 = ps_A.tile([128, F4], F32, tag="A", name="pq2")
                    for g in range(4):
                        mm(pq2[0:16, :F], w["ESO"][:, g * 16:(g + 1) * 16],
                           SQ24[:, g * F:(g + 1) * F], start=(g == 0), stop=(g == 3))
                    SSV = smpool.tile([128, F], F32, tag="ssv", name="ssv")
                    squash_scale(r32(SSV[:16, :F]), pq2[:16, :F], 16, F)

                    if it < 3:
                        SVS4 = vpool.tile([128, F4], F32, tag="svs4", name="svs4")
                        nc.scalar.activation(r32(SVS4[:]), psv4[:], AF.Copy)
                        pa4 = ps_A.tile([128, F4], F32, tag="A", name="pa4")
                        for g in range(4):
                            for m in range(2):
                                rr4 = ps_A.tile([128, F4], F32, tag="A", name="rr4")
                                for r in range(4):
                                    j = 4 * g + r
                                    sl = 32 * r
                                    mm(rr4[:, r * F:(r + 1) * F],
                                       w["WRR"][sl:sl + 16,
                                                j * 256 + m * 128: j * 256 + (m + 1) * 128],
                                       SVS4[sl:sl + 16, g * F:(g + 1) * F], tp=(sl, 0))
                                PR4 = qpool.tile([128, F4], F32, tag=f"q4{'ab'[m]}",
                                                 name=f"pr4{'ab'[m]}")
                                nc.vector.scalar_tensor_tensor(
                                    g3(r32(PR4[:])), g3(rr4[:]), 1.0,
                                    b4(P2[:, m * F:(m + 1) * F]),
                                    op0=ALU.mult, op1=ALU.mult)
                                for r in range(4):
                                    mm(pa4[:, g * F:(g + 1) * F],
                                       w["EADX"][:, (m * 4 + r) * 128:(m * 4 + r + 1) * 128],
                                       PR4[:, r * F:(r + 1) * F],
                                       start=(m == 0 and r == 0),
                                       stop=(m == 1 and r == 3))
                        pse4 = ps_A.tile([128, F4], F32, tag="A", name="pse4")
                        for q in range(4):
                            mm(pse4[:, q * F:(q + 1) * F],
                               w["ESV"][:, q * 128:(q + 1) * 128], SSV[:16, :F])
                        SE4 = smpool.tile([128, F4], F32, tag="se4", name="se4")
                        nc.scalar.activation(SE4[:], pse4[:], AF.Copy)
                        if it == 1:
                            nc.vector.scalar_tensor_tensor(
                                BLOG4[:], pa4[:], 1.0, SE4[:],
                                op0=ALU.mult, op1=ALU.mult)
                        else:
                            TA4 = smpool.tile([128, F4], F32, tag="ta4", name="ta4")
                            nc.vector.scalar_tensor_tensor(
                                TA4[:], pa4[:], 1.0, SE4[:],
                                op0=ALU.mult, op1=ALU.mult)
                            nc.vector.tensor_tensor(BLOG4[:], BLOG4[:], TA4[:], ALU.add)
                    else:
                        pv4 = ps_A.tile([128, F4], F32, tag="A", name="pv4")
                        for g in range(4):
                            mm(pv4[:, g * F:(g + 1) * F],
                               w["EVO"][:, g * 128:(g + 1) * 128], SSV[:16, :F])
                        SEV4 = smpool.tile([128, F4], F32, tag="se4", name="sev4")
                        nc.scalar.activation(SEV4[:], pv4[:], AF.Copy)
                        V4 = vpool.tile([128, F4], F32, tag="v4", name="v4")
                        nc.vector.scalar_tensor_tensor(
                            V4[:], psv4[:], 1.0, SEV4[:], op0=ALU.mult, op1=ALU.mult)
                        for g in range(4):
                            nc.sync.dma_start(out_d[g * 128:(g + 1) * 128, cols],
                                              V4[:, g * F:(g + 1) * F])

    nc.finalize()
    return nc


_NC_CACHE = {}


def _get_nc(n_core, F, reps=1):
    key = (n_core, F, reps)
    if key not in _NC_CACHE:
        _NC_CACHE[key] = build_module(n_core, F, reps)
    return _NC_CACHE[key]


_JIT_CACHE = {}


def get_jitted(nc):
    """Compile nc ONCE into a reusable 8-core jitted executable.

    run_bass_kernel_spmd re-creates (and re-compiles) a fresh jax.jit closure
    on every call, so each invocation pays XLA compile + NEFF reload on top of
    the actual execution.  Building the jitted callable once and reusing it
    makes repeat kernel() calls transfer+execute only.
    """
    if id(nc) in _JIT_CACHE:
        return _JIT_CACHE[id(nc)]
    import jax
    from jax.sharding import Mesh, PartitionSpec
    from jax.experimental.shard_map import shard_map
    from concourse import bass2jax
    from concourse.bass2jax import _bass_exec_p, install_neuronx_cc_hook

    install_neuronx_cc_hook()
    partition_name = nc.partition_id_tensor.name if nc.partition_id_tensor else None
    in_names, out_names, out_avals, out_shapes = [], [], [], []
    for alloc in nc.m.functions[0].allocations:
        if not isinstance(alloc, mybir.MemoryLocationSet):
            continue
        name = alloc.memorylocations[0].name
        if alloc.kind == "ExternalInput":
            if name != partition_name:
                in_names.append(name)
        elif alloc.kind == "ExternalOutput":
            shape = tuple(alloc.tensor_shape)
            dtype = mybir.dt.np(alloc.dtype)
            out_names.append(name)
            out_shapes.append((shape, dtype))
            out_avals.append(jax.core.ShapedArray(shape, dtype))
    n_params = len(in_names)
    all_in = list(in_names) + list(out_names)
    if partition_name is not None:
        all_in.append(partition_name)

    def _body(*args):
        operands = list(args)
        if partition_name is not None:
            operands.append(bass2jax.partition_id_tensor())
        outs = _bass_exec_p.bind(
            *operands, out_avals=tuple(out_avals), in_names=tuple(all_in),
            out_names=tuple(out_names), lowering_input_output_aliases=(),
            sim_require_finite=True, sim_require_nnan=True, nc=nc)
        return tuple(outs)

    donate = tuple(range(n_params, n_params + len(out_names)))
    devices = jax.devices()[:NCORES]
    mesh = Mesh(np.asarray(devices), ("core",))
    specs = (PartitionSpec("core"),) * (n_params + len(out_names))
    out_specs = (PartitionSpec("core"),) * len(out_names)
    jf = jax.jit(
        shard_map(_body, mesh=mesh, in_specs=specs, out_specs=out_specs,
                  check_rep=False),
        donate_argnums=donate, keep_unused=True)
    entry = dict(jf=jf, in_names=in_names, out_names=out_names,
                 out_shapes=out_shapes)
    _JIT_CACHE[id(nc)] = entry
    return entry


def run_jitted(nc, in_maps):
    """Execute via the cached jitted callable; returns {name: [per-core]}."""
    ent = get_jitted(nc)
    n = NCORES
    concat_in = [np.concatenate([np.asarray(in_maps[c][k]) for c in range(n)], axis=0)
                 for k in ent["in_names"]]
    zeros = [np.zeros((n * s[0], *s[1:]), d) for (s, d) in ent["out_shapes"]]
    out_arrs = ent["jf"](*concat_in, *zeros)
    res = {}
    for i, name in enumerate(ent["out_names"]):
        shape, _ = ent["out_shapes"][i]
        res[name] = np.asarray(out_arrs[i]).reshape(n, *shape)
    return res


def unpack_out(out, n):
    """device out [512, n] -> v [n, j, o]"""
    v = out.reshape(4, 4, 32, n)[:, :, :16, :]  # [nb, r, o, n]
    v = v.transpose(3, 0, 1, 2).reshape(n, 16, 16)  # [n, j=4nb+r, o]
    return v


def kernel(x, conv1_w, conv1_b, bn_gamma, bn_beta, bn_mean, bn_var, pc_w, pc_b, W_route):
    x = np.asarray(x, np.float32)
    weights = host_prep(
        np.asarray(conv1_w), np.asarray(conv1_b), np.asarray(bn_gamma),
        np.asarray(bn_beta), np.asarray(bn_mean), np.asarray(bn_var),
        np.asarray(pc_w), np.asarray(pc_b), np.asarray(W_route))

    xt = np.ascontiguousarray(x.reshape(NTOK, D).T)  # [256, 8192]
    nc = _get_nc(NCORE_TOK, 512)
    in_maps = []
    for c in range(NCORES):
        m = {"xt": np.ascontiguousarray(xt[:, c * NCORE_TOK:(c + 1) * NCORE_TOK])}
        m.update(weights)
        in_maps.append(m)
    res = run_jitted(nc, in_maps)
    v = np.concatenate(
        [unpack_out(res["out"][c], NCORE_TOK) for c in range(NCORES)], axis=0)
    out = np.swapaxes(v, 1, 2).reshape(B, S, OD * OC)  # [n, o, j] flattened
    return np.ascontiguousarray(out.astype(np.float32))

